# revision 50
# baseline (speedup 1.0000x reference)
"""Despawn2D (8-level db-style DWT analysis + synthesis) on 8 Trainium2 cores.

Math: the reference's FFT circular convolutions with 4-tap filters reduce to
4-tap circular stencils (L = 8192 is a power of two, so the ReplicationPad is
a no-op).  Per level:

  analysis:  out[j]  = f0*a[2j] + f1*a[2j-1] + f2*a[2j-2] + f3*a[2j-3] (mod N)
             with f = h (approx) and f = g (detail), g = flip(h)*(+,-,+,-)
  synthesis: out[2i]   = g0*d[i]   + g2*d[i+1] + h0*r[i]   + h2*r[i+1]
             out[2i+1] = g1*d[i+1] + g3*d[i+2] + h1*r[i+1] + h3*r[i+2] (mod m)

Each tap is one fused multiply-accumulate instruction (scalar_tensor_tensor)
over a [128, M] tile; circular wrap is handled with small halo regions.

Sharding: pure data parallel — 2048 rows / 8 cores = 256 rows/core,
processed as 2 tiles of 128 partitions x 8192.

When the provided filter bank is orthogonal (it is for the db2 filters the
reference uses), synthesis(analysis(x)) == x exactly, so the "rec" output is
produced by a DMA copy of the input tile and only the analysis runs on the
compute engines.  A host-side fp64 check of the perfect-reconstruction
property on a small probe vector selects that fast path; otherwise a full
on-device synthesis variant is used.
"""

import numpy as np

LEVELS = 8
L = 8192
ROWS_TOTAL = 2048
N_CORES = 8
RPC = ROWS_TOTAL // N_CORES  # rows per core
P = 128  # SBUF partitions
NT = RPC // P  # tiles per core

_nc_cache = {}


def _make_g(h):
    g = h[::-1].copy()
    g[1::2] *= -1.0
    return g


def _taps_array(scaling):
    """(LEVELS*8,) row: per level [h0..h3, g0..g3], tiled to (P, LEVELS*8)."""
    row = np.empty(LEVELS * 8, np.float32)
    for lev in range(LEVELS):
        h = scaling[lev].astype(np.float32)
        g = _make_g(h)
        row[lev * 8: lev * 8 + 4] = h
        row[lev * 8 + 4: lev * 8 + 8] = g
    return np.tile(row, (P, 1)).copy()


def _pr_is_identity(scaling):
    """fp64 host check: does synthesis(analysis(x)) == x for these filters?"""
    rng = np.random.default_rng(1234)
    n0 = 1 << (LEVELS + 2)
    x = rng.standard_normal((2, n0))
    a = x.copy()
    details = []
    for lev in range(LEVELS):
        h = scaling[lev].astype(np.float64)
        g = _make_g(h)
        N = a.shape[1]
        idx = (np.arange(N // 2)[:, None] * 2 - np.arange(4)[None, :]) % N
        d = (a[:, idx] * g).sum(-1)
        a = (a[:, idx] * h).sum(-1)
        details.append(d)
    r = a
    for lev in reversed(range(LEVELS)):
        h = scaling[lev].astype(np.float64)
        g = _make_g(h)
        d = details[lev]
        m = r.shape[1]
        out = np.empty((2, 2 * m))
        i = np.arange(m)
        out[:, 0::2] = (g[0] * d[:, i] + g[2] * d[:, (i + 1) % m]
                        + h[0] * r[:, i] + h[2] * r[:, (i + 1) % m])
        out[:, 1::2] = (g[1] * d[:, (i + 1) % m] + g[3] * d[:, (i + 2) % m]
                        + h[1] * r[:, (i + 1) % m] + h[3] * r[:, (i + 2) % m])
        r = out
    # scaling arrives as fp32, so an orthogonal filter bank reconstructs to
    # ~1e-8 (fp32 rounding of the filter constants), not fp64 precision.
    # Non-orthogonal filters give O(1) error, so 1e-6 separates cleanly.
    err = np.abs(r - x).max() / max(np.abs(x).max(), 1e-30)
    return err < 1e-6


def _build(synth: bool):
    import concourse.bacc as bacc
    import concourse.mybir as mybir
    from concourse.tile import TileContext

    f32 = mybir.dt.float32
    Alu = mybir.AluOpType

    nc = bacc.Bacc()
    x = nc.dram_tensor("x", [RPC, L], f32, kind="ExternalInput")
    taps = nc.dram_tensor("taps", [P, LEVELS * 8], f32, kind="ExternalInput")
    ident = nc.dram_tensor("ident", [P, P], f32, kind="ExternalInput")
    rec = nc.dram_tensor("rec", [RPC, L], f32, kind="ExternalOutput")
    coeffs = nc.dram_tensor("coeffs", [RPC, L], f32, kind="ExternalOutput")

    # detail block offsets inside a coeffs row: [d0 | d1 | ... | d7 | a8]
    doff = []
    off = 0
    for lev in range(LEVELS):
        doff.append(off)
        off += L >> (lev + 1)
    aoff = off  # 8160

    with TileContext(nc) as tc:
        import contextlib
        with contextlib.ExitStack() as ctx:
            cpool = ctx.enter_context(tc.tile_pool(name="consts", bufs=1))
            xpool = ctx.enter_context(
                tc.tile_pool(name="xio", bufs=1 if synth else 2))
            wpool = ctx.enter_context(
                tc.tile_pool(name="work", bufs=1 if synth else 2))
            dpool = ctx.enter_context(
                tc.tile_pool(name="dwork", bufs=1 if synth else 2))
            spool = ctx.enter_context(tc.tile_pool(name="sc", bufs=1))
            ppool = ctx.enter_context(
                tc.tile_pool(name="psum", bufs=6, space="PSUM"))

            tp = cpool.tile([P, LEVELS * 8], f32)
            nc.sync.dma_start(out=tp[:, :], in_=taps[:, :])

            def tap(lev, k):  # h taps
                c = lev * 8 + k
                return tp[:, c:c + 1]

            def gtap(lev, k):  # g taps
                c = lev * 8 + 4 + k
                return tp[:, c:c + 1]

            # scaled identities: diag(tap) as matmul weights, so some conv
            # taps can run on the (otherwise idle) tensor engine and
            # accumulate in PSUM.  fp32 matmul streams at 4 cyc/col (PE ~77
            # Gmac/s vs DVE 123), so the PE tap count per level is tuned to
            # balance the two engines.
            PE_TAPS = {lev: 3 for lev in range(LEVELS)}
            id_t = cpool.tile([P, P], f32)
            nc.sync.dma_start(out=id_t[:, :], in_=ident[:, :])
            sid = {}
            for lev, ntap in PE_TAPS.items():
                for k in range(ntap):
                    t = cpool.tile([P, P], f32, tag=f"sid_g{lev}{k}")
                    nc.scalar.mul(t[:, :], id_t[:, :], gtap(lev, k))
                    sid[("g", lev, k)] = t

            # Each input row is loaded as two half tiles so level-0 compute
            # can start after half the load.  In ext coordinates
            # (ext[i] == a[i-3]): xlo holds ext[0 .. 3+Nh) (3-elem circular
            # halo + first half), xhi holds ext[Nh .. 3+L) (3-elem overlap +
            # second half).  The hi DMA is issued first since it carries the
            # wrap halo source.
            Nh = L // 2
            Mh = Nh // 2
            xts = []
            for t in range(NT):
                rows = slice(t * P, (t + 1) * P)
                xlo = xpool.tile([P, 3 + Nh], f32, tag="xlo")
                xhi = xpool.tile([P, 3 + Nh], f32, tag="xhi")
                nc.sync.dma_start(out=xhi[:, 0:3 + Nh], in_=x[rows, Nh - 3:L])
                nc.sync.dma_start(out=xlo[:, 3:3 + Nh], in_=x[rows, 0:Nh])
                nc.vector.tensor_copy(
                    out=xlo[:, 0:3], in_=xhi[:, Nh:Nh + 3])
                if not synth:
                    # orthogonal filter bank: synthesis(analysis(x)) == x
                    nc.sync.dma_start(
                        out=rec[rows, 0:Nh], in_=xlo[:, 3:3 + Nh])
                    nc.sync.dma_start(
                        out=rec[rows, Nh:L], in_=xhi[:, 3:3 + Nh])
                xts.append((xlo, xhi))

            # ---------------- analysis ----------------
            # levels outer, tiles inner: the two tiles' serial approx chains
            # are independent, so interleaving them lets tile B's DVE work
            # fill tile A's ScalarE/PE handoff bubbles (and vice versa).
            a_exts = list(xts)  # ext[i] == a[i-3]
            d_tiles_all = [[] for _ in range(NT)]
            a_lasts = [None] * NT
            if synth:
                # tile-sequential: bufs=1 tags can't interleave tiles
                order = [(lev, t) for t in range(NT) for lev in range(LEVELS)]
            else:
                order = [(lev, t) for lev in range(LEVELS) for t in range(NT)]
            for lev, t in order:
                if True:
                    rows = slice(t * P, (t + 1) * P)
                    N = L >> lev
                    M = N >> 1
                    last = lev == LEVELS - 1
                    # (jbase, source tile, source ext-width) — level 0 reads
                    # the two half tiles, all other levels one full buffer
                    if lev == 0:
                        halves = ((0, xts[t][0], Nh), (Mh, xts[t][1], Nh))
                    else:
                        halves = ((0, a_exts[t], N),)
                    if not last:
                        a_t = wpool.tile([P, M + 3], f32, tag=f"a{lev}")
                        a_main = a_t[:, 3:3 + M]
                    else:
                        a_t = wpool.tile([P, M + 2], f32, tag=f"a{lev}")
                        a_main = a_t[:, 0:M]
                    d_t = dpool.tile([P, M + 2], f32, tag=f"d{lev}")
                    d_main = d_t[:, 0:M]

                    # tap k of output j reads ext[(3-k) + 2j].
                    # approx chain (critical path): first tap on ScalarE,
                    # remaining three as in-place fused MACs on DVE.
                    # (a 4-engine ACT/GPS/DVE split of this chain measured
                    # WORSE: GpSimd's latency inside the per-level serial
                    # chain cost more in lost overlap than it saved in
                    # DVE busy time)
                    pe_ntap = PE_TAPS.get(lev, 0)
                    for jb, src, W in halves:
                        W2 = W >> 1
                        am = a_main[:, jb:jb + W2]
                        nc.scalar.mul(am, src[:, 3:3 + W:2], tap(lev, 0))
                        for k in (1, 2, 3):
                            nc.vector.scalar_tensor_tensor(
                                out=am,
                                in0=src[:, 3 - k:3 - k + W:2],
                                scalar=tap(lev, k),
                                in1=am,
                                op0=Alu.mult,
                                op1=Alu.add,
                            )
                    if pe_ntap:
                        # detail chain: taps 0..pe_ntap-1 as diag matmuls
                        # accumulated in PSUM; the first remaining tap is
                        # fused with the PSUM evacuation on DVE (in1 reads
                        # PSUM directly), the rest accumulate in place.
                        for c0 in range(0, M, 512):
                            F = min(512, M - c0)
                            jb, src, W = halves[-1] if c0 >= (M >> 1) and \
                                lev == 0 else halves[0]
                            cl = c0 - jb  # chunk offset within the source
                            ps = ppool.tile([P, F], f32, tag="pg")
                            for k in range(pe_ntap):
                                off = (3 - k) + 2 * cl
                                nc.tensor.matmul(
                                    ps[:, 0:F],
                                    sid[("g", lev, k)][:, :],
                                    src[:, off:off + 2 * F:2],
                                    start=(k == 0),
                                    stop=(k == pe_ntap - 1),
                                )
                            for k in range(pe_ntap, 4):
                                off = (3 - k) + 2 * cl
                                nc.vector.scalar_tensor_tensor(
                                    out=d_main[:, c0:c0 + F],
                                    in0=src[:, off:off + 2 * F:2],
                                    scalar=gtap(lev, k),
                                    in1=(ps[:, 0:F] if k == pe_ntap
                                         else d_main[:, c0:c0 + F]),
                                    op0=Alu.mult,
                                    op1=Alu.add,
                                )
                    else:
                        for jb, src, W in halves:
                            W2 = W >> 1
                            dm = d_main[:, jb:jb + W2]
                            nc.scalar.mul(dm, src[:, 3:3 + W:2], gtap(lev, 0))
                            for k in (1, 2, 3):
                                nc.vector.scalar_tensor_tensor(
                                    out=dm,
                                    in0=src[:, 3 - k:3 - k + W:2],
                                    scalar=gtap(lev, k),
                                    in1=dm,
                                    op0=Alu.mult,
                                    op1=Alu.add,
                                )

                    # details go straight out to HBM
                    nc.sync.dma_start(
                        out=coeffs[rows, doff[lev]:doff[lev] + M], in_=d_main)
                    if last:
                        nc.sync.dma_start(
                            out=coeffs[rows, aoff:aoff + M], in_=a_main)

                    if not last:
                        # left halo: ext[0:3] = a[M-3:M]
                        nc.vector.tensor_copy(
                            out=a_t[:, 0:3], in_=a_t[:, M:M + 3])
                    elif synth:
                        # right halo for synthesis start
                        nc.vector.tensor_copy(
                            out=a_t[:, M:M + 2], in_=a_t[:, 0:2])
                    d_tiles_all[t].append(d_t)
                    a_exts[t] = a_t
                    if last:
                        a_lasts[t] = a_t

            # ---------------- synthesis ----------------
            if synth:
                for t in range(NT):
                    rows = slice(t * P, (t + 1) * P)
                    xlo, xhi = xts[t]
                    d_tiles = d_tiles_all[t]
                    r_ext = a_lasts[t]  # [P, m+2] with right halo
                    for lev in reversed(range(LEVELS)):
                        m = L >> (lev + 1)
                        d_t = d_tiles[lev]
                        # fill d right halo: d[m:m+2] = d[0:2]
                        nc.vector.tensor_copy(
                            out=d_t[:, m:m + 2], in_=d_t[:, 0:2])
                        h4 = [tap(lev, k) for k in range(4)]
                        g4 = [gtap(lev, k) for k in range(4)]
                        if lev > 0:
                            o_t = wpool.tile([P, 2 * m + 2], f32, tag=f"r{lev}")
                            parts = ((0, m, o_t[:, 0:2 * m:2],
                                      o_t[:, 1:2 * m:2]),)
                        else:
                            # final level: write into the xlo/xhi halves,
                            # then DMA them to rec
                            mh = m // 2
                            parts = (
                                (0, mh, xlo[:, 3:3 + Nh:2], xlo[:, 4:3 + Nh:2]),
                                (mh, mh, xhi[:, 3:3 + Nh:2], xhi[:, 4:3 + Nh:2]),
                            )
                        for ib, w, ev, od in parts:
                            # even: g0*d[i] + g2*d[i+1] + h0*r[i] + h2*r[i+1]
                            nc.vector.tensor_scalar_mul(
                                ev, d_t[:, ib:ib + w], g4[0])
                            for src, s in (
                                    (d_t[:, ib + 1:ib + w + 1], g4[2]),
                                    (r_ext[:, ib:ib + w], h4[0]),
                                    (r_ext[:, ib + 1:ib + w + 1], h4[2])):
                                nc.vector.scalar_tensor_tensor(
                                    out=ev, in0=src, scalar=s, in1=ev,
                                    op0=Alu.mult, op1=Alu.add)
                            # odd: g1*d[i+1] + g3*d[i+2]
                            #      + h1*r[i+1] + h3*r[i+2]
                            nc.vector.tensor_scalar_mul(
                                od, d_t[:, ib + 1:ib + w + 1], g4[1])
                            for src, s in (
                                    (d_t[:, ib + 2:ib + w + 2], g4[3]),
                                    (r_ext[:, ib + 1:ib + w + 1], h4[1]),
                                    (r_ext[:, ib + 2:ib + w + 2], h4[3])):
                                nc.vector.scalar_tensor_tensor(
                                    out=od, in0=src, scalar=s, in1=od,
                                    op0=Alu.mult, op1=Alu.add)
                        if lev > 0:
                            nc.vector.tensor_copy(
                                out=o_t[:, 2 * m:2 * m + 2], in_=o_t[:, 0:2])
                            r_ext = o_t
                    nc.sync.dma_start(out=rec[rows, 0:Nh], in_=xlo[:, 3:3 + Nh])
                    nc.sync.dma_start(out=rec[rows, Nh:L], in_=xhi[:, 3:3 + Nh])

    nc.finalize()
    return nc


def _get_nc(synth: bool):
    key = ("synth", synth)
    if key not in _nc_cache:
        _nc_cache[key] = _build(synth)
    return _nc_cache[key]


def kernel(x: np.ndarray, scaling: np.ndarray):
    from concourse.bass_utils import run_bass_kernel_spmd

    x = np.ascontiguousarray(np.asarray(x, np.float32))
    scaling = np.asarray(scaling, np.float32)
    assert x.shape == (ROWS_TOTAL, L), x.shape
    assert scaling.shape == (LEVELS, 4), scaling.shape

    synth = not _pr_is_identity(scaling)
    nc = _get_nc(synth)

    taps = _taps_array(scaling)
    ident = np.eye(P, dtype=np.float32)
    in_maps = [
        {"x": np.ascontiguousarray(x[i * RPC:(i + 1) * RPC]), "taps": taps,
         "ident": ident}
        for i in range(N_CORES)
    ]
    res = None
    last_err = None
    for attempt in range(3):
        try:
            res = run_bass_kernel_spmd(
                nc, in_maps, core_ids=list(range(N_CORES)))
            break
        except Exception as e:  # transient NRT device wedge: retry
            last_err = e
    if res is None:
        raise last_err
    outs = res.results
    rec = np.concatenate([outs[i]["rec"] for i in range(N_CORES)], axis=0)
    coeffs = np.concatenate([outs[i]["coeffs"] for i in range(N_CORES)], axis=0)
    return rec, coeffs
